# revision 17
# baseline (speedup 1.0000x reference)
"""Trainium2 Bass kernel for nn_Block_38285338477091 (dense transformer block).

Strategy:
- Data-parallel over batch: 8 NeuronCores, one batch element [1024,1024] each,
  weights replicated, zero collectives.
- LoRA folded into the base weights host-side (x@W + ((x@A)@B)*s == x@(W + s*A@B)).
- fp8 (e4m3) DoubleRow matmuls for the attention side: qkv/v/PV/proj weights
  prescaled x64 and cast to fp8 host-side; LN1 output, v, softmax probs and
  attention outputs stored as fp8 contraction-PAIR tiles [128, 2, N] so each
  DoubleRow matmul contracts 256 rows at ~2x throughput. Scale compensated at
  PSUM evictions (q/k/v x1/64) and the proj residual (scalar_tensor_tensor).
  m1/m2 stay bf16 (fp8 there breaches the accuracy budget: each fp8 operand
  alone adds ~1.2e-2 mean rel err vs the 2e-2 gate).
- Attention software-pipelined at emission level: q/k for head-pair p+1 are
  computed while scores/exp/PV for pair p run, so the per-pair dependency
  chain (qk matmul -> evict -> scores -> exp -> PV -> normalize) overlaps
  across pairs on all engines.
- Causal attention: scores via 64-row head pairs (PE row-tiling), softmax
  without max-subtraction, per-column sums ride PV as a 65th ones-column of v;
  causal-masked pair columns zeroed in the fp8 prob tiles (mask multiply on
  the Pool engine) so PV DoubleRow matmuls run full width exactly; k>q blocks
  skipped.
- LN transpose evictions split between DVE and Act engines.
- h ([4096,1024] bf16) spills through a DRAM scratch between m1 and m2.
- m2_b is added host-side (exact, final linear term).
"""
import numpy as np
import ml_dtypes
from contextlib import ExitStack

import concourse.bass as bass
import concourse.tile as tile
from concourse import bacc, mybir
from concourse.bass_utils import run_bass_kernel_spmd  # noqa: F401 (fallback path)

f32 = mybir.dt.float32
f32r = mybir.dt.float32r
bf16 = mybir.dt.bfloat16
fp8 = mybir.dt.float8e4
DR = mybir.MatmulPerfMode.DoubleRow
WS = 64.0
FT = mybir.ActivationFunctionType
OP = mybir.AluOpType

P = 128
T = 1024
C = 1024
NHEAD = 16
HD = 64
EPS = 1e-5
N_CORES = 8

_NC_CACHE = {}


def build_nc(reps=1):
    if ("nc", reps) in _NC_CACHE:
        return _NC_CACHE[("nc", reps)]
    nc = bacc.Bacc("TRN2", target_bir_lowering=False, debug=False)

    d_btc = nc.dram_tensor("btc", [T, C], f32, kind="ExternalInput").ap()
    d_wqkv = nc.dram_tensor("wqkv", [C, 3 * C], fp8, kind="ExternalInput").ap()
    d_wm1 = nc.dram_tensor("wm1", [C, 4 * C], bf16, kind="ExternalInput").ap()
    d_wproj = nc.dram_tensor("wproj", [C, C], fp8, kind="ExternalInput").ap()
    d_wm2 = nc.dram_tensor("wm2", [4 * C, C], bf16, kind="ExternalInput").ap()
    d_ln1g = nc.dram_tensor("ln1g", [P, 8], f32, kind="ExternalInput").ap()
    d_ln1b = nc.dram_tensor("ln1b", [P, 8], f32, kind="ExternalInput").ap()
    d_ln2g = nc.dram_tensor("ln2g", [P, 8], f32, kind="ExternalInput").ap()
    d_ln2b = nc.dram_tensor("ln2b", [P, 8], f32, kind="ExternalInput").ap()
    d_m1b = nc.dram_tensor("m1b", [P, 32], f32, kind="ExternalInput").ap()
    d_ident = nc.dram_tensor("ident", [P, P], bf16, kind="ExternalInput").ap()
    d_cmask = nc.dram_tensor("cmask", [4, P, 512], bf16, kind="ExternalInput").ap()

    d_out = nc.dram_tensor("out", [T, C], f32, kind="ExternalOutput").ap()
    d_hs = nc.dram_tensor("h_scratch", [4 * C, T], bf16)  # internal DRAM

    with tile.TileContext(nc) as tc, ExitStack() as ctx:
        consts = ctx.enter_context(tc.tile_pool(name="consts", bufs=1))
        btcp = ctx.enter_context(tc.tile_pool(name="btcp", bufs=1))
        xnp = ctx.enter_context(tc.tile_pool(name="xnp", bufs=3))
        lnTp = ctx.enter_context(tc.tile_pool(name="lnTp", bufs=1))
        qkp = ctx.enter_context(tc.tile_pool(name="qkp", bufs=4))
        vp = ctx.enter_context(tc.tile_pool(name="vp", bufs=1))
        aop = ctx.enter_context(tc.tile_pool(name="aop", bufs=1))
        esp = ctx.enter_context(tc.tile_pool(name="esp", bufs=9))
        wq = ctx.enter_context(tc.tile_pool(name="wq", bufs=3))
        wb = ctx.enter_context(tc.tile_pool(name="wb", bufs=4))
        wv = ctx.enter_context(tc.tile_pool(name="wv", bufs=1))
        hcp = ctx.enter_context(tc.tile_pool(name="hcp", bufs=10))
        htp = ctx.enter_context(tc.tile_pool(name="htp", bufs=2))
        rp = ctx.enter_context(tc.tile_pool(name="rp", bufs=2))
        stp = ctx.enter_context(tc.tile_pool(name="stp", bufs=4))
        pp = ctx.enter_context(tc.tile_pool(name="pp", bufs=8, space="PSUM"))
        for _rep in range(reps):

          # ---- constants first (ident gates the LN1 transposes) ----
          ident = consts.tile([P, P], bf16, tag="ident")
          nc.sync.dma_start(ident[:], d_ident[:])
          dmask = consts.tile([P, P], bf16, tag="dmask")
          nc.sync.dma_start(dmask[:], d_cmask[0][:, 0:P])
          dmask8 = consts.tile([P, P], fp8, tag="dmask8")
          nc.gpsimd.tensor_copy(dmask8[:], dmask[:])
          g1 = consts.tile([P, 8], f32, tag="g1")
          b1 = consts.tile([P, 8], f32, tag="b1")
          g2 = consts.tile([P, 8], f32, tag="g2")
          b2 = consts.tile([P, 8], f32, tag="b2")
          m1b = consts.tile([P, 32], f32, tag="m1b")
          for t_, d_ in ((g1, d_ln1g), (b1, d_ln1b), (g2, d_ln2g), (b2, d_ln2b),
                         (m1b, d_m1b)):
              nc.sync.dma_start(t_[:], d_[:])
          epst = consts.tile([P, 1], f32, tag="epst")
          nc.vector.memset(epst[:], EPS)

          # ---- btc loads split across the SP and Act DGE queues ----
          btc_tiles = []
          for i in range(8):
              bt = btcp.tile([P, C], f32, tag=f"bt{i}")
              nc.sync.dma_start(bt[:], d_btc[P * i:P * (i + 1), :])
              btc_tiles.append(bt)

          # ---- helpers ----
          def layernorm_tile(x_tile):
              """token-major LN: returns normalized (x-mu)*rstd tile."""
              bn6 = stp.tile([P, 2, 6], f32, tag="bn6")
              nc.vector.bn_stats(bn6[:, 0, :], x_tile[:, 0:512])
              nc.vector.bn_stats(bn6[:, 1, :], x_tile[:, 512:1024])
              mv = stp.tile([P, 2], f32, tag="mv")
              nc.vector.bn_aggr(mv[:], bn6[:])
              sq = stp.tile([P, 1], f32, tag="sq")
              nc.scalar.activation(sq[:], mv[:, 1:2], FT.Sqrt, bias=epst[:])
              rstd = stp.tile([P, 1], f32, tag="rstd")
              nc.vector.reciprocal(rstd[:], sq[:])
              xn = xnp.tile([P, T], bf16, tag="xn")
              nc.gpsimd.tensor_scalar(out=xn[:], in0=x_tile[:],
                                      scalar1=mv[:, 0:1], scalar2=rstd[:],
                                      op0=OP.subtract, op1=OP.mult)
              return xn

          def transpose_ln(x_tiles, dst_fn, g_sb, b_sb):
              """x (token-major) -> dst_fn(i, j) = x^T block with g/b.
              Evictions alternate between DVE and Act."""
              for i in range(8):
                  xn = x_tiles[i]
                  for j in range(8):
                      trp = pp.tile([P, P], bf16, tag="ps")
                      nc.tensor.transpose(trp[:], xn[:, P * j:P * (j + 1)], ident[:])
                      nc.vector.tensor_scalar(
                          out=dst_fn(i, j), in0=trp[:],
                          scalar1=g_sb[:, j:j + 1], scalar2=b_sb[:, j:j + 1],
                          op0=OP.mult, op1=OP.add)

          # ---- phase 1: LN1, transpose -> fp8 kb-pair tiles xl2 ----
          xl2 = [lnTp.tile([P, 2, T], fp8, tag=f"t{j2}", name=f"xl2_{j2}")
                 for j2 in range(4)]

          def xl_dst(i, j):
              return xl2[j // 2][:, j % 2, P * i:P * (i + 1)]

          xn_tiles = []
          for i in range(8):
              bt = btc_tiles[i]
              xn_tiles.append(layernorm_tile(bt))
          transpose_ln(xn_tiles, xl_dst, g1, b1)

          # ---- phase 2a: v (token-major) fp8 kt-pair tiles with ones cols ----
          v2t = []
          for ktp in range(4):
              vt = vp.tile([P, 2, 1040], fp8, tag=f"v{ktp}")
              vt4 = vt[:].rearrange("p two (h d) -> p two h d", d=65)
              nc.vector.memset(vt4[:, :, :, 64:65], 1.0)
              v2t.append(vt)
          wqkv3 = d_wqkv.rearrange("(kb p) c -> p kb c", p=P)
          wm13 = d_wm1.rearrange("(kb p) c -> p kb c", p=P)
          wproj3 = d_wproj.rearrange("(kb p) c -> p kb c", p=P)
          wm23 = d_wm2.rearrange("(kb p) c -> p kb c", p=P)
          wvch = wv.tile([P, 8, T], fp8, tag="wv")
          nc.sync.dma_start(wvch[:], wqkv3[:, :, 2 * C:3 * C])
          for tt in range(8):
              pss = [pp.tile([P, 512], f32, tag="ps", name=f"vps{tt}_{cb}")
                     for cb in range(2)]
              for t in range(4):
                  for cb in range(2):
                      nc.tensor.matmul(pss[cb][:],
                                       xl2[t][:, :, P * tt:P * (tt + 1)],
                                       wvch[:, 2 * t:2 * t + 2,
                                            512 * cb:512 * (cb + 1)],
                                       perf_mode=DR,
                                       start=(t == 0), stop=(t == 3))
              vt3 = v2t[tt // 2][:, tt % 2].rearrange("p (h d) -> p h d", d=65)
              for cb in range(2):
                  nc.scalar.activation(
                      vt3[:, 8 * cb:8 * (cb + 1), 0:64],
                      pss[cb][:].rearrange("p (h d) -> p h d", d=64),
                      FT.Copy, scale=1.0 / WS)

          # ---- phases 2b/3: software-pipelined attention ----
          qk_tiles = {}

          def qk_compute(p):
              for which, co in (("q", p), ("k", 8 + p)):
                  dst = qkp.tile([P, T], bf16, tag=which, name=f"{which}T{p}")
                  wch = wq.tile([P, 8, P], fp8, tag="wqk", name=f"w{which}{p}")
                  nc.sync.dma_start(
                      wch[:], wqkv3[:, :, P * co:P * (co + 1)])
                  pss = [pp.tile([P, 512], f32, tag="ps", name=f"qk{p}{which}{qb}")
                         for qb in range(2)]
                  for t in range(4):
                      for qb in range(2):
                          nc.tensor.matmul(
                              pss[qb][:], wch[:, 2 * t:2 * t + 2, :],
                              xl2[t][:, :, 512 * qb:512 * (qb + 1)],
                              perf_mode=DR,
                              start=(t == 0), stop=(t == 3))
                  for qb in range(2):
                      nc.vector.tensor_scalar(
                          out=dst[:, 512 * qb:512 * (qb + 1)], in0=pss[qb][:],
                          scalar1=1.0 / WS, scalar2=None, op0=OP.mult)
                  qk_tiles[(which, p)] = dst

          ao2 = [aop.tile([P, 2, T], fp8, tag=f"ao{p2}", name=f"ao2_{p2}")
                 for p2 in range(4)]

          def attention(p):
              qT, kT = qk_tiles[("q", p)], qk_tiles[("k", p)]
              # heads a=2p (rows 0:64) and 2p+1 (rows 64:128); both q-blocks
              # processed jointly so kT / v_aug stationaries load once each.
              pv_ps = {(qb, u): pp.tile([65, 512], f32, tag="ps",
                                        name=f"pv{p}_{qb}_{u}")
                       for qb in range(2) for u in range(2)}

              def emit_exp(es, s_ps, mi):
                  if 0 <= mi < 4:
                      d0 = 128 * mi
                      nc.scalar.activation(es[:, d0:512], s_ps[:, d0:512],
                                           FT.Exp, scale=0.125)
                      nc.vector.tensor_tensor(out=es[:, d0:d0 + P],
                                              in0=es[:, d0:d0 + P],
                                              in1=dmask8[:], op=OP.mult)
                  else:
                      nc.scalar.activation(es[:], s_ps[:], FT.Exp, scale=0.125)

              for ck in range(0, 8, 2):
                  ckp = ck // 2
                  ess = {}
                  for kt in range(ck, ck + 2):
                      for u in range(2):
                          rows = slice(64 * u, 64 * (u + 1))
                          for qb in range(2):
                              if kt >= 4 * (qb + 1):
                                  continue
                              mi = kt - 4 * qb
                              d0 = 128 * mi if 0 <= mi < 4 else 0
                              s_ps = pp.tile([P, 512], f32, tag="ps")
                              nc.tensor.matmul(
                                  s_ps[:, d0:512], kT[rows, P * kt:P * (kt + 1)],
                                  qT[rows, 512 * qb + d0:512 * (qb + 1)],
                                  start=True, stop=True)
                              if (u, qb) not in ess:
                                  e2n = esp.tile(
                                      [P, 2, 512], fp8, tag="es",
                                      name=f"es{p}_{ck}_{u}_{qb}")
                                  ess[(u, qb)] = e2n
                                  mi0 = ck - 4 * qb
                                  if 0 <= mi0 < 4:
                                      # masked pair: zero plane1's dead cols so
                                      # the full-width DR matmul is exact
                                      nc.gpsimd.memset(
                                          e2n[:, 1, 128 * mi0:128 * (mi0 + 1)],
                                          0.0)
                              pl = ess[(u, qb)][:, kt - ck, :]
                              emit_exp(pl, s_ps, mi)
                  for u in range(2):
                      a = 2 * p + u
                      for qb in range(2):
                          if ck >= 4 * (qb + 1):
                              continue
                          d0l = 128 * (ck - 4 * qb) if 0 <= ck - 4 * qb < 4 else 0
                          e2 = ess[(u, qb)]
                          nc.tensor.matmul(
                              pv_ps[(qb, u)][:, d0l:512],
                              v2t[ckp][:, :, 65 * a:65 * a + 65],
                              e2[:, :, d0l:512], perf_mode=DR,
                              start=(ck == 0), stop=(ck >= 4 * (qb + 1) - 2))
              for qb in range(2):
                  for u in range(2):
                      rrow = rp.tile([1, 512], f32, tag="rrow")
                      nc.vector.reciprocal(rrow[:], pv_ps[(qb, u)][64:65, :])
                      R = rp.tile([64, 512], f32, tag="R")
                      nc.gpsimd.partition_broadcast(R[:], rrow[:])
                      nc.vector.tensor_tensor(
                          out=ao2[p // 2][64 * u:64 * (u + 1), p % 2,
                                          512 * qb:512 * (qb + 1)],
                          in0=pv_ps[(qb, u)][0:64, :], in1=R[:], op=OP.mult)

          for p in range(9):
              if p < 8:
                  qk_compute(p)
              if p >= 1:
                  attention(p - 1)

          # ---- phase 4: proj (token-major) + residual into btc tiles ----
          prch = {}
          for cb in range(2):
              w = wb.tile([P, 8, 512], fp8, tag="wb", name=f"wproj{cb}")
              nc.sync.dma_start(w[:], wproj3[:, :, 512 * cb:512 * (cb + 1)])
              prch[cb] = w
          for tt in range(8):
              pss = [pp.tile([P, 512], f32, tag="ps", name=f"prps{tt}_{cb}")
                     for cb in range(2)]
              for t in range(4):
                  for cb in range(2):
                      nc.tensor.matmul(pss[cb][:],
                                       ao2[t][:, :, P * tt:P * (tt + 1)],
                                       prch[cb][:, 2 * t:2 * t + 2, :],
                                       perf_mode=DR,
                                       start=(t == 0), stop=(t == 3))
              for cb in range(2):
                  sl = btc_tiles[tt][:, 512 * cb:512 * (cb + 1)]
                  nc.vector.scalar_tensor_tensor(
                      out=sl, in0=pss[cb][:], scalar=1.0 / WS, in1=sl,
                      op0=OP.mult, op1=OP.add)

          # ---- phase 5: LN2 + transpose -> ylnT (reuses lnT tags) ----
          yn_tiles = []
          for i in range(8):
              yn_tiles.append(layernorm_tile(btc_tiles[i]))
          ylnT = [lnTp.tile([P, T], bf16,
                            tag=(f"t{j}" if j < 4 else f"y{j}"),
                            name=f"ylnT{j}")
                  for j in range(8)]

          def yl_dst(i, j):
              return ylnT[j][:, P * i:P * (i + 1)]

          transpose_ln(yn_tiles, yl_dst, g2, b2)

          # ---- phase 6: m1 + GELU -> h_scratch (DRAM) ----
          for co in range(32):
              wch = wq.tile([P, 8, P], bf16, tag="wqk", name=f"wm1_{co}")
              nc.sync.dma_start(wch[:], wm13[:, :, P * co:P * (co + 1)])
              ht = htp.tile([P, T], bf16, tag="ht")
              pss = [pp.tile([P, 512], f32, tag="ps", name=f"m1ps{co}_{qb}")
                     for qb in range(2)]
              for kb in range(8):
                  for qb in range(2):
                      nc.tensor.matmul(pss[qb][:], wch[:, kb, :],
                                       ylnT[kb][:, 512 * qb:512 * (qb + 1)],
                                       start=(kb == 0), stop=(kb == 7))
              for qb in range(2):
                  nc.scalar.activation(ht[:, 512 * qb:512 * (qb + 1)], pss[qb][:],
                                       FT.Gelu, bias=m1b[:, co:co + 1])
              nc.sync.dma_start(d_hs[P * co:P * (co + 1), :], ht[:])

          # ---- phase 7: m2 (token-major) + bias + residual -> out ----
          for quarter in range(4):
              wch = {}
              for cb in range(2):
                  w = wb.tile([P, 8, 512], bf16, tag="wb", name=f"wm2_{quarter}{cb}")
                  nc.sync.dma_start(
                      w[:], wm23[:, 8 * quarter:8 * (quarter + 1),
                                 512 * cb:512 * (cb + 1)])
                  wch[cb] = w
              for ttg in range(2):
                  hcs = []
                  for kb_l in range(8):
                      kb = 8 * quarter + kb_l
                      hc = hcp.tile([P, 512], bf16, tag="hc")
                      nc.sync.dma_start(
                          hc[:], d_hs[P * kb:P * (kb + 1), 512 * ttg:512 * (ttg + 1)])
                      hcs.append(hc)
                  for tt_l in range(4):
                      tt = 4 * ttg + tt_l
                      pss = [pp.tile([P, 512], f32, tag="ps",
                                     name=f"m2ps{quarter}_{tt}_{cb}")
                             for cb in range(2)]
                      for kb_l in range(8):
                          for cb in range(2):
                              nc.tensor.matmul(
                                  pss[cb][:], hcs[kb_l][:, P * tt_l:P * (tt_l + 1)],
                                  wch[cb][:, kb_l, :],
                                  start=(kb_l == 0), stop=(kb_l == 7))
                      for cb in range(2):
                          sl = btc_tiles[tt][:, 512 * cb:512 * (cb + 1)]
                          nc.vector.tensor_tensor(out=sl, in0=pss[cb][:], in1=sl,
                                                  op=OP.add)
                      if quarter == 3:
                          nc.sync.dma_start(d_out[P * tt:P * (tt + 1), :],
                                            btc_tiles[tt][:])

    nc.compile()
    _NC_CACHE[("nc", reps)] = nc
    return nc


def _prep_inputs(inputs):
    """Host-side preprocessing: LoRA fold, rearrange, casts, constants."""
    f = np.float32
    qkv_w = np.asarray(inputs["qkv_w"], f)
    m1_w = np.asarray(inputs["m1_w"], f)
    m2_w = np.asarray(inputs["m2_w"], f)
    s = 1.0 / 16.0
    wqkv = qkv_w + s * (np.asarray(inputs["qkv_A"], f) @ np.asarray(inputs["qkv_B"], f))
    wm1 = m1_w + s * (np.asarray(inputs["m1_A"], f) @ np.asarray(inputs["m1_B"], f))
    wm2 = m2_w + s * (np.asarray(inputs["m2_A"], f) @ np.asarray(inputs["m2_B"], f))

    def re_pc(v, n):  # [n*128] -> [128, n] with c = 128*j + p
        return np.ascontiguousarray(np.asarray(v, f).reshape(n, P).T)

    masks = np.zeros((4, P, 512), f)
    qi = np.arange(512)[None, :]
    ki = np.arange(P)[:, None]
    for m in range(4):
        masks[m] = (qi - ki >= 128 * m).astype(f)

    def to8(w):
        return np.clip(w * 64.0, -240.0, 240.0).astype(ml_dtypes.float8_e4m3)

    common = {
        "wqkv": to8(wqkv),
        "wm1": wm1.astype(ml_dtypes.bfloat16),
        "wproj": to8(np.asarray(inputs["proj_w"], f)),
        "wm2": wm2.astype(ml_dtypes.bfloat16),
        "ln1g": re_pc(inputs["ln1_g"], 8),
        "ln1b": re_pc(inputs["ln1_b"], 8),
        "ln2g": re_pc(inputs["ln2_g"], 8),
        "ln2b": re_pc(inputs["ln2_b"], 8),
        "m1b": re_pc(inputs["m1_b"], 32),
        "ident": np.eye(P).astype(ml_dtypes.bfloat16),
        "cmask": masks.astype(ml_dtypes.bfloat16),
    }
    btc = np.asarray(inputs["btc"], f)
    in_maps = [dict(common, btc=np.ascontiguousarray(btc[c]))
               for c in range(N_CORES)]
    return in_maps


def _make_runner(nc):
    """Persistent sharded jit over the 8 cores (cached across kernel() calls)."""
    import jax
    from jax.sharding import Mesh, PartitionSpec
    from jax.experimental.shard_map import shard_map
    from concourse.bass2jax import (_bass_exec_p, install_neuronx_cc_hook,
                                    partition_id_tensor)
    install_neuronx_cc_hook()
    in_names, out_names, out_avals, zero_outs = [], [], [], []
    for alloc in nc.m.functions[0].allocations:
        if not isinstance(alloc, mybir.MemoryLocationSet):
            continue
        name = alloc.memorylocations[0].name
        if alloc.kind == "ExternalInput":
            in_names.append(name)
        elif alloc.kind == "ExternalOutput":
            out_names.append(name)
            shape = tuple(alloc.tensor_shape)
            dtype = mybir.dt.np(alloc.dtype)
            out_avals.append(jax.core.ShapedArray(shape, dtype))
            zero_outs.append(np.zeros(shape, dtype))
    pname = nc.partition_id_tensor.name if nc.partition_id_tensor else None
    if pname is not None and pname in in_names:
        in_names.remove(pname)
    n_params = len(in_names)
    all_in_names = in_names + out_names + ([pname] if pname else [])

    def _body(*args):
        operands = list(args)
        if pname is not None:
            operands.append(partition_id_tensor())
        outs = _bass_exec_p.bind(
            *operands, out_avals=tuple(out_avals), in_names=tuple(all_in_names),
            out_names=tuple(out_names), lowering_input_output_aliases=(),
            sim_require_finite=True, sim_require_nnan=True, nc=nc)
        return tuple(outs)

    devices = jax.devices()[:N_CORES]
    mesh = Mesh(np.asarray(devices), ("core",))
    specs = (PartitionSpec("core"),) * (n_params + len(out_names))
    fn = jax.jit(shard_map(_body, mesh=mesh, in_specs=specs,
                           out_specs=(PartitionSpec("core"),) * len(out_names),
                           check_rep=False), keep_unused=True)

    def run(in_maps):
        args = []
        for name in in_names:
            args.append(np.concatenate([np.asarray(m[name]) for m in in_maps],
                                       axis=0))
        for z in zero_outs:
            args.append(np.zeros((N_CORES * z.shape[0], *z.shape[1:]), z.dtype))
        out_arrs = fn(*args)
        return [
            {name: np.asarray(out_arrs[i]).reshape(N_CORES, *out_avals[i].shape)[c]
             for i, name in enumerate(out_names)}
            for c in range(N_CORES)]

    return run


def kernel(**inputs):
    nc = build_nc()
    if "runner" not in _NC_CACHE:
        _NC_CACHE["runner"] = _make_runner(nc)
    in_maps = _prep_inputs(inputs)
    results = _NC_CACHE["runner"](in_maps)
    out = np.stack([results[c]["out"] for c in range(N_CORES)])
    out += np.asarray(inputs["m2_b"], np.float32)[None, None, :]
    return out.astype(np.float32)


# revision 18
# speedup vs baseline: 1.1975x; 1.1975x over previous
"""Trainium2 Bass kernel for nn_Block_38285338477091 (dense transformer block).

Strategy:
- Data-parallel over batch: 8 NeuronCores, one batch element [1024,1024] each,
  weights replicated, zero collectives.
- LoRA folded into the base weights host-side (x@W + ((x@A)@B)*s == x@(W + s*A@B)).
- fp8 (e4m3) DoubleRow matmuls for the attention side: qkv/v/PV/proj weights
  prescaled x64 and cast to fp8 host-side; LN1 output, v, softmax probs and
  attention outputs stored as fp8 contraction-PAIR tiles [128, 2, N] so each
  DoubleRow matmul contracts 256 rows at ~2x throughput. Scale compensated at
  PSUM evictions (q/k/v x1/64) and the proj residual (scalar_tensor_tensor).
  m1/m2 stay bf16 (fp8 there breaches the accuracy budget: each fp8 operand
  alone adds ~1.2e-2 mean rel err vs the 2e-2 gate).
- Attention software-pipelined at emission level: q/k for head-pair p+1 are
  computed while scores/exp/PV for pair p run, so the per-pair dependency
  chain (qk matmul -> evict -> scores -> exp -> PV -> normalize) overlaps
  across pairs on all engines.
- Causal attention: scores via 64-row head pairs (PE row-tiling), softmax
  without max-subtraction, per-column sums ride PV as a 65th ones-column of v;
  causal-masked pair columns zeroed in the fp8 prob tiles so PV DoubleRow
  matmuls run full width exactly; k>q blocks skipped.
- h ([4096,1024] bf16) spills through a DRAM scratch between m1 and m2.
- m2_b is added host-side (exact, final linear term).
"""
import numpy as np
import ml_dtypes
from contextlib import ExitStack

import concourse.bass as bass
import concourse.tile as tile
from concourse import bacc, mybir
from concourse.bass_utils import run_bass_kernel_spmd  # noqa: F401 (fallback path)

f32 = mybir.dt.float32
f32r = mybir.dt.float32r
bf16 = mybir.dt.bfloat16
fp8 = mybir.dt.float8e4
DR = mybir.MatmulPerfMode.DoubleRow
WS = 64.0
FT = mybir.ActivationFunctionType
OP = mybir.AluOpType

P = 128
T = 1024
C = 1024
NHEAD = 16
HD = 64
EPS = 1e-5
N_CORES = 8

_NC_CACHE = {}


def build_nc(reps=1):
    if ("nc", reps) in _NC_CACHE:
        return _NC_CACHE[("nc", reps)]
    nc = bacc.Bacc("TRN2", target_bir_lowering=False, debug=False)

    d_btc = nc.dram_tensor("btc", [T, C], f32, kind="ExternalInput").ap()
    d_wqkv = nc.dram_tensor("wqkv", [C, 3 * C], fp8, kind="ExternalInput").ap()
    d_wm1 = nc.dram_tensor("wm1", [C, 4 * C], bf16, kind="ExternalInput").ap()
    d_wproj = nc.dram_tensor("wproj", [C, C], fp8, kind="ExternalInput").ap()
    d_wm2 = nc.dram_tensor("wm2", [4 * C, C], bf16, kind="ExternalInput").ap()
    d_ln1g = nc.dram_tensor("ln1g", [P, 8], f32, kind="ExternalInput").ap()
    d_ln1b = nc.dram_tensor("ln1b", [P, 8], f32, kind="ExternalInput").ap()
    d_ln2g = nc.dram_tensor("ln2g", [P, 8], f32, kind="ExternalInput").ap()
    d_ln2b = nc.dram_tensor("ln2b", [P, 8], f32, kind="ExternalInput").ap()
    d_m1b = nc.dram_tensor("m1b", [P, 32], f32, kind="ExternalInput").ap()
    d_ident = nc.dram_tensor("ident", [P, P], bf16, kind="ExternalInput").ap()
    d_cmask = nc.dram_tensor("cmask", [4, P, 512], bf16, kind="ExternalInput").ap()

    d_out = nc.dram_tensor("out", [T, C], f32, kind="ExternalOutput").ap()
    d_hs = nc.dram_tensor("h_scratch", [4 * C, T], bf16)  # internal DRAM

    with tile.TileContext(nc) as tc, ExitStack() as ctx:
        consts = ctx.enter_context(tc.tile_pool(name="consts", bufs=1))
        btcp = ctx.enter_context(tc.tile_pool(name="btcp", bufs=1))
        xnp = ctx.enter_context(tc.tile_pool(name="xnp", bufs=3))
        lnTp = ctx.enter_context(tc.tile_pool(name="lnTp", bufs=1))
        qkp = ctx.enter_context(tc.tile_pool(name="qkp", bufs=4))
        vp = ctx.enter_context(tc.tile_pool(name="vp", bufs=1))
        aop = ctx.enter_context(tc.tile_pool(name="aop", bufs=1))
        esp = ctx.enter_context(tc.tile_pool(name="esp", bufs=9))
        wq = ctx.enter_context(tc.tile_pool(name="wq", bufs=3))
        wb = ctx.enter_context(tc.tile_pool(name="wb", bufs=4))
        wv = ctx.enter_context(tc.tile_pool(name="wv", bufs=1))
        hcp = ctx.enter_context(tc.tile_pool(name="hcp", bufs=10))
        htp = ctx.enter_context(tc.tile_pool(name="htp", bufs=2))
        rp = ctx.enter_context(tc.tile_pool(name="rp", bufs=2))
        stp = ctx.enter_context(tc.tile_pool(name="stp", bufs=4))
        pp = ctx.enter_context(tc.tile_pool(name="pp", bufs=8, space="PSUM"))
        for _rep in range(reps):

          # ---- constants first (ident gates the LN1 transposes) ----
          ident = consts.tile([P, P], bf16, tag="ident")
          nc.sync.dma_start(ident[:], d_ident[:])
          dmask = consts.tile([P, P], bf16, tag="dmask")
          nc.sync.dma_start(dmask[:], d_cmask[0][:, 0:P])
          dmask8 = consts.tile([P, P], fp8, tag="dmask8")
          nc.gpsimd.tensor_copy(dmask8[:], dmask[:])
          g1 = consts.tile([P, 8], f32, tag="g1")
          b1 = consts.tile([P, 8], f32, tag="b1")
          g2 = consts.tile([P, 8], f32, tag="g2")
          b2 = consts.tile([P, 8], f32, tag="b2")
          m1b = consts.tile([P, 32], f32, tag="m1b")
          for t_, d_ in ((g1, d_ln1g), (b1, d_ln1b), (g2, d_ln2g), (b2, d_ln2b),
                         (m1b, d_m1b)):
              nc.sync.dma_start(t_[:], d_[:])
          epst = consts.tile([P, 1], f32, tag="epst")
          nc.vector.memset(epst[:], EPS)

          # ---- btc loads (consts precede them so ident lands early) ----
          btc_tiles = []
          for i in range(8):
              bt = btcp.tile([P, C], f32, tag=f"bt{i}")
              nc.sync.dma_start(bt[:], d_btc[P * i:P * (i + 1), :])
              btc_tiles.append(bt)

          # ---- helpers ----
          def layernorm_tile(x_tile):
              """token-major LN: returns normalized (x-mu)*rstd tile."""
              bn6 = stp.tile([P, 2, 6], f32, tag="bn6")
              nc.vector.bn_stats(bn6[:, 0, :], x_tile[:, 0:512])
              nc.vector.bn_stats(bn6[:, 1, :], x_tile[:, 512:1024])
              mv = stp.tile([P, 2], f32, tag="mv")
              nc.vector.bn_aggr(mv[:], bn6[:])
              sq = stp.tile([P, 1], f32, tag="sq")
              nc.scalar.activation(sq[:], mv[:, 1:2], FT.Sqrt, bias=epst[:])
              rstd = stp.tile([P, 1], f32, tag="rstd")
              nc.vector.reciprocal(rstd[:], sq[:])
              xn = xnp.tile([P, T], bf16, tag="xn")
              nc.gpsimd.tensor_scalar(out=xn[:], in0=x_tile[:],
                                      scalar1=mv[:, 0:1], scalar2=rstd[:],
                                      op0=OP.subtract, op1=OP.mult)
              return xn

          def transpose_ln(x_tiles, dst_fn, g_sb, b_sb):
              """x (token-major) -> dst_fn(i, j) = x^T block with g/b."""
              for i in range(8):
                  xn = x_tiles[i]
                  for j in range(8):
                      trp = pp.tile([P, P], bf16, tag="ps")
                      nc.tensor.transpose(trp[:], xn[:, P * j:P * (j + 1)], ident[:])
                      nc.vector.tensor_scalar(
                          out=dst_fn(i, j), in0=trp[:],
                          scalar1=g_sb[:, j:j + 1], scalar2=b_sb[:, j:j + 1],
                          op0=OP.mult, op1=OP.add)

          # ---- phase 1: LN1, transpose -> fp8 kb-pair tiles xl2 ----
          xl2 = [lnTp.tile([P, 2, T], fp8, tag=f"t{j2}", name=f"xl2_{j2}")
                 for j2 in range(4)]

          def xl_dst(i, j):
              return xl2[j // 2][:, j % 2, P * i:P * (i + 1)]

          xn_tiles = []
          for i in range(8):
              bt = btc_tiles[i]
              xn_tiles.append(layernorm_tile(bt))
          transpose_ln(xn_tiles, xl_dst, g1, b1)

          # ---- phase 2a: v (token-major) fp8 kt-pair tiles with ones cols ----
          v2t = []
          for ktp in range(4):
              vt = vp.tile([P, 2, 1040], fp8, tag=f"v{ktp}")
              vt4 = vt[:].rearrange("p two (h d) -> p two h d", d=65)
              nc.vector.memset(vt4[:, :, :, 64:65], 1.0)
              v2t.append(vt)
          wqkv3 = d_wqkv.rearrange("(kb p) c -> p kb c", p=P)
          wm13 = d_wm1.rearrange("(kb p) c -> p kb c", p=P)
          wproj3 = d_wproj.rearrange("(kb p) c -> p kb c", p=P)
          wm23 = d_wm2.rearrange("(kb p) c -> p kb c", p=P)
          wvch = wv.tile([P, 8, T], fp8, tag="wv")
          nc.sync.dma_start(wvch[:], wqkv3[:, :, 2 * C:3 * C])
          for tt in range(8):
              pss = [pp.tile([P, 512], f32, tag="ps", name=f"vps{tt}_{cb}")
                     for cb in range(2)]
              for t in range(4):
                  for cb in range(2):
                      nc.tensor.matmul(pss[cb][:],
                                       xl2[t][:, :, P * tt:P * (tt + 1)],
                                       wvch[:, 2 * t:2 * t + 2,
                                            512 * cb:512 * (cb + 1)],
                                       perf_mode=DR,
                                       start=(t == 0), stop=(t == 3))
              vt3 = v2t[tt // 2][:, tt % 2].rearrange("p (h d) -> p h d", d=65)
              for cb in range(2):
                  nc.scalar.activation(
                      vt3[:, 8 * cb:8 * (cb + 1), 0:64],
                      pss[cb][:].rearrange("p (h d) -> p h d", d=64),
                      FT.Copy, scale=1.0 / WS)

          # ---- phases 2b/3: software-pipelined attention ----
          qk_tiles = {}

          def qk_compute(p):
              for which, co in (("q", p), ("k", 8 + p)):
                  dst = qkp.tile([P, T], bf16, tag=which, name=f"{which}T{p}")
                  wch = wq.tile([P, 8, P], fp8, tag="wqk", name=f"w{which}{p}")
                  nc.sync.dma_start(
                      wch[:], wqkv3[:, :, P * co:P * (co + 1)])
                  pss = [pp.tile([P, 512], f32, tag="ps", name=f"qk{p}{which}{qb}")
                         for qb in range(2)]
                  for t in range(4):
                      for qb in range(2):
                          nc.tensor.matmul(
                              pss[qb][:], wch[:, 2 * t:2 * t + 2, :],
                              xl2[t][:, :, 512 * qb:512 * (qb + 1)],
                              perf_mode=DR,
                              start=(t == 0), stop=(t == 3))
                  for qb in range(2):
                      nc.vector.tensor_scalar(
                          out=dst[:, 512 * qb:512 * (qb + 1)], in0=pss[qb][:],
                          scalar1=1.0 / WS, scalar2=None, op0=OP.mult)
                  qk_tiles[(which, p)] = dst

          ao2 = [aop.tile([P, 2, T], fp8, tag=f"ao{p2}", name=f"ao2_{p2}")
                 for p2 in range(4)]

          def attention(p):
              qT, kT = qk_tiles[("q", p)], qk_tiles[("k", p)]
              # heads a=2p (rows 0:64) and 2p+1 (rows 64:128); both q-blocks
              # processed jointly so kT / v_aug stationaries load once each.
              pv_ps = {(qb, u): pp.tile([65, 512], f32, tag="ps",
                                        name=f"pv{p}_{qb}_{u}")
                       for qb in range(2) for u in range(2)}

              def emit_exp(es, s_ps, mi):
                  if 0 <= mi < 4:
                      d0 = 128 * mi
                      nc.scalar.activation(es[:, d0:512], s_ps[:, d0:512],
                                           FT.Exp, scale=0.125)
                      nc.vector.tensor_tensor(out=es[:, d0:d0 + P],
                                              in0=es[:, d0:d0 + P],
                                              in1=dmask8[:], op=OP.mult)
                  else:
                      nc.scalar.activation(es[:], s_ps[:], FT.Exp, scale=0.125)

              for ck in range(0, 8, 2):
                  ckp = ck // 2
                  ess = {}
                  for kt in range(ck, ck + 2):
                      for u in range(2):
                          rows = slice(64 * u, 64 * (u + 1))
                          for qb in range(2):
                              if kt >= 4 * (qb + 1):
                                  continue
                              mi = kt - 4 * qb
                              d0 = 128 * mi if 0 <= mi < 4 else 0
                              s_ps = pp.tile([P, 512], f32, tag="ps")
                              nc.tensor.matmul(
                                  s_ps[:, d0:512], kT[rows, P * kt:P * (kt + 1)],
                                  qT[rows, 512 * qb + d0:512 * (qb + 1)],
                                  start=True, stop=True)
                              if (u, qb) not in ess:
                                  e2n = esp.tile(
                                      [P, 2, 512], fp8, tag="es",
                                      name=f"es{p}_{ck}_{u}_{qb}")
                                  ess[(u, qb)] = e2n
                                  mi0 = ck - 4 * qb
                                  if 0 <= mi0 < 4:
                                      # masked pair: zero plane1's dead cols so
                                      # the full-width DR matmul is exact
                                      nc.gpsimd.memset(
                                          e2n[:, 1, 128 * mi0:128 * (mi0 + 1)],
                                          0.0)
                              pl = ess[(u, qb)][:, kt - ck, :]
                              emit_exp(pl, s_ps, mi)
                  for u in range(2):
                      a = 2 * p + u
                      for qb in range(2):
                          if ck >= 4 * (qb + 1):
                              continue
                          d0l = 128 * (ck - 4 * qb) if 0 <= ck - 4 * qb < 4 else 0
                          e2 = ess[(u, qb)]
                          nc.tensor.matmul(
                              pv_ps[(qb, u)][:, d0l:512],
                              v2t[ckp][:, :, 65 * a:65 * a + 65],
                              e2[:, :, d0l:512], perf_mode=DR,
                              start=(ck == 0), stop=(ck >= 4 * (qb + 1) - 2))
              for qb in range(2):
                  for u in range(2):
                      rrow = rp.tile([1, 512], f32, tag="rrow")
                      nc.vector.reciprocal(rrow[:], pv_ps[(qb, u)][64:65, :])
                      R = rp.tile([64, 512], f32, tag="R")
                      nc.gpsimd.partition_broadcast(R[:], rrow[:])
                      nc.vector.tensor_tensor(
                          out=ao2[p // 2][64 * u:64 * (u + 1), p % 2,
                                          512 * qb:512 * (qb + 1)],
                          in0=pv_ps[(qb, u)][0:64, :], in1=R[:], op=OP.mult)

          for p in range(9):
              if p < 8:
                  qk_compute(p)
              if p >= 1:
                  attention(p - 1)

          # ---- phase 4: proj (token-major) + residual into btc tiles ----
          prch = {}
          for cb in range(2):
              w = wb.tile([P, 8, 512], fp8, tag="wb", name=f"wproj{cb}")
              nc.sync.dma_start(w[:], wproj3[:, :, 512 * cb:512 * (cb + 1)])
              prch[cb] = w
          for tt in range(8):
              pss = [pp.tile([P, 512], f32, tag="ps", name=f"prps{tt}_{cb}")
                     for cb in range(2)]
              for t in range(4):
                  for cb in range(2):
                      nc.tensor.matmul(pss[cb][:],
                                       ao2[t][:, :, P * tt:P * (tt + 1)],
                                       prch[cb][:, 2 * t:2 * t + 2, :],
                                       perf_mode=DR,
                                       start=(t == 0), stop=(t == 3))
              for cb in range(2):
                  sl = btc_tiles[tt][:, 512 * cb:512 * (cb + 1)]
                  nc.vector.scalar_tensor_tensor(
                      out=sl, in0=pss[cb][:], scalar=1.0 / WS, in1=sl,
                      op0=OP.mult, op1=OP.add)

          # ---- phase 5: LN2 + transpose -> ylnT (reuses lnT tags) ----
          yn_tiles = []
          for i in range(8):
              yn_tiles.append(layernorm_tile(btc_tiles[i]))
          ylnT = [lnTp.tile([P, T], bf16,
                            tag=(f"t{j}" if j < 4 else f"y{j}"),
                            name=f"ylnT{j}")
                  for j in range(8)]

          def yl_dst(i, j):
              return ylnT[j][:, P * i:P * (i + 1)]

          transpose_ln(yn_tiles, yl_dst, g2, b2)

          # ---- phase 6: m1 + GELU -> h_scratch (DRAM) ----
          for co in range(32):
              wch = wq.tile([P, 8, P], bf16, tag="wqk", name=f"wm1_{co}")
              nc.sync.dma_start(wch[:], wm13[:, :, P * co:P * (co + 1)])
              ht = htp.tile([P, T], bf16, tag="ht")
              pss = [pp.tile([P, 512], f32, tag="ps", name=f"m1ps{co}_{qb}")
                     for qb in range(2)]
              for kb in range(8):
                  for qb in range(2):
                      nc.tensor.matmul(pss[qb][:], wch[:, kb, :],
                                       ylnT[kb][:, 512 * qb:512 * (qb + 1)],
                                       start=(kb == 0), stop=(kb == 7))
              for qb in range(2):
                  nc.scalar.activation(ht[:, 512 * qb:512 * (qb + 1)], pss[qb][:],
                                       FT.Gelu, bias=m1b[:, co:co + 1])
              nc.sync.dma_start(d_hs[P * co:P * (co + 1), :], ht[:])

          # ---- phase 7: m2 (token-major) + bias + residual -> out ----
          for quarter in range(4):
              wch = {}
              for cb in range(2):
                  w = wb.tile([P, 8, 512], bf16, tag="wb", name=f"wm2_{quarter}{cb}")
                  nc.sync.dma_start(
                      w[:], wm23[:, 8 * quarter:8 * (quarter + 1),
                                 512 * cb:512 * (cb + 1)])
                  wch[cb] = w
              for ttg in range(2):
                  hcs = []
                  for kb_l in range(8):
                      kb = 8 * quarter + kb_l
                      hc = hcp.tile([P, 512], bf16, tag="hc")
                      nc.sync.dma_start(
                          hc[:], d_hs[P * kb:P * (kb + 1), 512 * ttg:512 * (ttg + 1)])
                      hcs.append(hc)
                  for tt_l in range(4):
                      tt = 4 * ttg + tt_l
                      pss = [pp.tile([P, 512], f32, tag="ps",
                                     name=f"m2ps{quarter}_{tt}_{cb}")
                             for cb in range(2)]
                      for kb_l in range(8):
                          for cb in range(2):
                              nc.tensor.matmul(
                                  pss[cb][:], hcs[kb_l][:, P * tt_l:P * (tt_l + 1)],
                                  wch[cb][:, kb_l, :],
                                  start=(kb_l == 0), stop=(kb_l == 7))
                      for cb in range(2):
                          sl = btc_tiles[tt][:, 512 * cb:512 * (cb + 1)]
                          nc.vector.tensor_tensor(out=sl, in0=pss[cb][:], in1=sl,
                                                  op=OP.add)
                      if quarter == 3:
                          nc.sync.dma_start(d_out[P * tt:P * (tt + 1), :],
                                            btc_tiles[tt][:])

    nc.compile()
    _NC_CACHE[("nc", reps)] = nc
    return nc


def _prep_inputs(inputs):
    """Host-side preprocessing: LoRA fold, rearrange, casts, constants."""
    f = np.float32
    qkv_w = np.asarray(inputs["qkv_w"], f)
    m1_w = np.asarray(inputs["m1_w"], f)
    m2_w = np.asarray(inputs["m2_w"], f)
    s = 1.0 / 16.0
    wqkv = qkv_w + s * (np.asarray(inputs["qkv_A"], f) @ np.asarray(inputs["qkv_B"], f))
    wm1 = m1_w + s * (np.asarray(inputs["m1_A"], f) @ np.asarray(inputs["m1_B"], f))
    wm2 = m2_w + s * (np.asarray(inputs["m2_A"], f) @ np.asarray(inputs["m2_B"], f))

    def re_pc(v, n):  # [n*128] -> [128, n] with c = 128*j + p
        return np.ascontiguousarray(np.asarray(v, f).reshape(n, P).T)

    masks = np.zeros((4, P, 512), f)
    qi = np.arange(512)[None, :]
    ki = np.arange(P)[:, None]
    for m in range(4):
        masks[m] = (qi - ki >= 128 * m).astype(f)

    def to8(w):
        return np.clip(w * 64.0, -240.0, 240.0).astype(ml_dtypes.float8_e4m3)

    common = {
        "wqkv": to8(wqkv),
        "wm1": wm1.astype(ml_dtypes.bfloat16),
        "wproj": to8(np.asarray(inputs["proj_w"], f)),
        "wm2": wm2.astype(ml_dtypes.bfloat16),
        "ln1g": re_pc(inputs["ln1_g"], 8),
        "ln1b": re_pc(inputs["ln1_b"], 8),
        "ln2g": re_pc(inputs["ln2_g"], 8),
        "ln2b": re_pc(inputs["ln2_b"], 8),
        "m1b": re_pc(inputs["m1_b"], 32),
        "ident": np.eye(P).astype(ml_dtypes.bfloat16),
        "cmask": masks.astype(ml_dtypes.bfloat16),
    }
    btc = np.asarray(inputs["btc"], f)
    in_maps = [dict(common, btc=np.ascontiguousarray(btc[c]))
               for c in range(N_CORES)]
    return in_maps


def _make_runner(nc):
    """Persistent sharded jit over the 8 cores (cached across kernel() calls)."""
    import jax
    from jax.sharding import Mesh, PartitionSpec
    from jax.experimental.shard_map import shard_map
    from concourse.bass2jax import (_bass_exec_p, install_neuronx_cc_hook,
                                    partition_id_tensor)
    install_neuronx_cc_hook()
    in_names, out_names, out_avals, zero_outs = [], [], [], []
    for alloc in nc.m.functions[0].allocations:
        if not isinstance(alloc, mybir.MemoryLocationSet):
            continue
        name = alloc.memorylocations[0].name
        if alloc.kind == "ExternalInput":
            in_names.append(name)
        elif alloc.kind == "ExternalOutput":
            out_names.append(name)
            shape = tuple(alloc.tensor_shape)
            dtype = mybir.dt.np(alloc.dtype)
            out_avals.append(jax.core.ShapedArray(shape, dtype))
            zero_outs.append(np.zeros(shape, dtype))
    pname = nc.partition_id_tensor.name if nc.partition_id_tensor else None
    if pname is not None and pname in in_names:
        in_names.remove(pname)
    n_params = len(in_names)
    all_in_names = in_names + out_names + ([pname] if pname else [])

    def _body(*args):
        operands = list(args)
        if pname is not None:
            operands.append(partition_id_tensor())
        outs = _bass_exec_p.bind(
            *operands, out_avals=tuple(out_avals), in_names=tuple(all_in_names),
            out_names=tuple(out_names), lowering_input_output_aliases=(),
            sim_require_finite=True, sim_require_nnan=True, nc=nc)
        return tuple(outs)

    devices = jax.devices()[:N_CORES]
    mesh = Mesh(np.asarray(devices), ("core",))
    specs = (PartitionSpec("core"),) * (n_params + len(out_names))
    fn = jax.jit(shard_map(_body, mesh=mesh, in_specs=specs,
                           out_specs=(PartitionSpec("core"),) * len(out_names),
                           check_rep=False), keep_unused=True)

    def run(in_maps):
        args = []
        for name in in_names:
            args.append(np.concatenate([np.asarray(m[name]) for m in in_maps],
                                       axis=0))
        for z in zero_outs:
            args.append(np.zeros((N_CORES * z.shape[0], *z.shape[1:]), z.dtype))
        out_arrs = fn(*args)
        return [
            {name: np.asarray(out_arrs[i]).reshape(N_CORES, *out_avals[i].shape)[c]
             for i, name in enumerate(out_names)}
            for c in range(N_CORES)]

    return run


def kernel(**inputs):
    nc = build_nc()
    if "runner" not in _NC_CACHE:
        _NC_CACHE["runner"] = _make_runner(nc)
    in_maps = _prep_inputs(inputs)
    results = _NC_CACHE["runner"](in_maps)
    out = np.stack([results[c]["out"] for c in range(N_CORES)])
    out += np.asarray(inputs["m2_b"], np.float32)[None, None, :]
    return out.astype(np.float32)
